# revision 15
# baseline (speedup 1.0000x reference)
"""Strided (residue-group) attention for Trainium2, SPMD across 8 NeuronCores.

Problem: x[B=2,S=4096,E=1024] -> qkv proj -> per-(batch,head,residue-group)
attention (stride 8 -> 8 groups of n=512 tokens) -> out proj.

Sharding: by (batch, residue-group).  B*stride = 16 group-instances; each of
the 8 cores owns 2 (batch,group) pairs = 1024 tokens and computes their FULL
output rows (it holds all 16 heads for its tokens).  The residue groups are
independent, so there are no cross-device collectives at all; the host
permutes tokens into group-major order on the way in and inverts on the way
out.

Device kernel design (per core), v2:
  - All weights (wq/wk/wv/wo) SBUF-resident in fp16; startup DMAs split into
    <=128KB pieces so they spread across the 16 DMA queues.
  - QKV: qT,kT produced feature-on-partition ([f,tok]); v produced
    token-on-partition ([tok,f]).
  - v tiles [128tok, 8 pairs x 192]: per head-pair block {v_even(64) |
    ones(64) | v_odd(64)} with the ones written once by memset.  PV lhsT is
    a contiguous 128-col slice: cols 0-127 for the even head (o lands at
    PSUM rows 0-63), cols 64-191 for the odd head (o at rows 64-127) --
    matching the outproj lhsT row layout with no cross-partition moves,
    while 64 copies of the softmax denominator fill the other row half for
    free (matmul cost only depends on the moving free size).
  - scoresT[k,q] = kT.T-chunks @ qT per head; head pairs are row-packed on
    the PE array (K=64 each at array rows 0-63 / 64-127).
  - exp on ScalarE without max-subtraction (scores are O(+-8), exp is safe).
    EXP is the only table the ScalarE ever loads (no ln-based reciprocal
    anymore -> no activation-table thrash).
  - softmax denominators: one row per head gathered (ScalarE copy) into a
    [16, 512] f32 tile per group; ONE nc.vector.reciprocal_approx_fast per
    group; replication DMA spreads each recip row across 64 partitions; DVE
    multiplies normalize o.
  - out proj: lhsT = oT chunks, rhs = resident Wout rows -> [tok, E] fp16
    output, DMA'd out in partition-split pieces (4-way for the final unit so
    the tail DMA is short).
"""

import os

import numpy as np

B, S, E = 2, 4096, 1024
H, ST = 16, 8
DH = E // H  # 64
N = S // ST  # 512 tokens per residue group
NCORES = 8
GPC = (B * ST) // NCORES  # 2 (batch,group) pairs per core
TOK = GPC * N  # 1024 tokens per core
P = 128
EC = E // P  # 8 contraction chunks of 128
NB = N // P  # 4 token chunks of 128 per group
FB = 2  # feature blocks of 512 in E
SCALE = 1.0 / float(np.sqrt(DH))

_CACHE: dict = {}


def _build_nc():
    import concourse.bass as bass
    import concourse.bacc as bacc
    import concourse.tile as tile
    from concourse import mybir

    F32 = mybir.dt.float32
    FP16 = mybir.dt.float16
    ADD = mybir.AluOpType.add
    EXP = mybir.ActivationFunctionType.Exp

    PRW = 3 * DH  # per-pair v block width: v_even | ones | v_odd
    VW = EC * PRW  # v tile free width (8 pairs x 192)

    nc = bacc.Bacc()
    xt = nc.declare_dram_parameter("xt", [E, TOK], FP16, isOutput=False)
    wq = nc.declare_dram_parameter("wq", [EC, P, EC, P], FP16, isOutput=False)
    wk = nc.declare_dram_parameter("wk", [EC, P, EC, P], FP16, isOutput=False)
    wv = nc.declare_dram_parameter("wv", [EC, P, E], FP16, isOutput=False)
    wo = nc.declare_dram_parameter("wo", [E, E], FP16, isOutput=False)
    bq = nc.declare_dram_parameter("bq", [E], F32, isOutput=False)
    bk = nc.declare_dram_parameter("bk", [E], F32, isOutput=False)
    bv = nc.declare_dram_parameter("bv", [E], F32, isOutput=False)
    bo = nc.declare_dram_parameter("bo", [E], F32, isOutput=False)
    out = nc.declare_dram_parameter("out", [TOK, E], FP16, isOutput=True)

    with tile.TileContext(nc) as tc, (
        tc.tile_pool(name="const", bufs=1)
    ) as const, tc.tile_pool(name="xtp", bufs=1) as xtp, tc.tile_pool(
        name="wqp", bufs=1
    ) as wqp, tc.tile_pool(name="wkp", bufs=1) as wkp, tc.tile_pool(
        name="wvp", bufs=1
    ) as wvp, tc.tile_pool(name="qtp", bufs=9) as qtp, tc.tile_pool(
        name="ktp", bufs=9
    ) as ktp, tc.tile_pool(name="vpp", bufs=8) as vpp, tc.tile_pool(
        name="expp", bufs=4
    ) as expp, tc.tile_pool(name="otp", bufs=16) as otp, tc.tile_pool(
        name="recp", bufs=4
    ) as recp, tc.tile_pool(name="outp", bufs=3) as outp, tc.tile_pool(
        name="osbp", bufs=18
    ) as osbp, tc.tile_pool(name="psmm", bufs=2, space="PSUM") as psmm, tc.tile_pool(
        name="pssc", bufs=2, space="PSUM"
    ) as pssc, tc.tile_pool(name="pso", bufs=2, space="PSUM") as psop:
        # ---- persistent tiles ------------------------------------------
        bq_sb = const.tile([P, EC], F32)
        bk_sb = const.tile([P, EC], F32)
        bv_bc = const.tile([P, E], F32)
        bo_bc = const.tile([P, E], F32)
        wo_sb = const.tile([P, FB, EC, 512], FP16)
        wq_sb = [wqp.tile([P, EC, P], FP16, name=f"wq{ft}") for ft in range(EC)]
        wk_sb = [wkp.tile([P, EC, P], FP16, name=f"wk{ft}") for ft in range(EC)]
        wv_sb = [wvp.tile([P, E], FP16, name=f"wv{c}") for c in range(EC)]
        xt_t = [xtp.tile([P, TOK], FP16, name=f"xt{c}") for c in range(EC)]
        denb = [const.tile([H, N], F32, name=f"denb{g}") for g in range(GPC)]
        recb = [const.tile([H, N], F32, name=f"recb{g}") for g in range(GPC)]

        # ---- startup DMAs (ordered for earliest first matmul) ----------
        nc.sync.dma_start(out=bq_sb, in_=bq[:].rearrange("(c p) -> p c", p=P))
        nc.sync.dma_start(out=bk_sb, in_=bk[:].rearrange("(c p) -> p c", p=P))
        # wq ftile 0 split in halves, then group-0 xt halves: these gate the
        # first q-projection chain.
        nc.sync.dma_start(out=wq_sb[0][:, 0:4, :], in_=wq[0][:, 0:4, :])
        nc.sync.dma_start(out=wq_sb[0][:, 4:8, :], in_=wq[0][:, 4:8, :])
        for c in range(EC):
            nc.sync.dma_start(
                out=xt_t[c][:, 0:N], in_=xt[c * P : (c + 1) * P, 0:N]
            )
        nc.sync.dma_start(out=wk_sb[0][:, 0:4, :], in_=wk[0][:, 0:4, :])
        nc.sync.dma_start(out=wk_sb[0][:, 4:8, :], in_=wk[0][:, 4:8, :])
        for ft in range(1, EC):
            nc.sync.dma_start(out=wq_sb[ft][:, 0:4, :], in_=wq[ft][:, 0:4, :])
            nc.sync.dma_start(out=wq_sb[ft][:, 4:8, :], in_=wq[ft][:, 4:8, :])
            nc.sync.dma_start(out=wk_sb[ft][:, 0:4, :], in_=wk[ft][:, 0:4, :])
            nc.sync.dma_start(out=wk_sb[ft][:, 4:8, :], in_=wk[ft][:, 4:8, :])
        for c in range(EC):
            nc.sync.dma_start(
                out=xt_t[c][:, N:TOK], in_=xt[c * P : (c + 1) * P, N:TOK]
            )
        nc.gpsimd.dma_start(out=bv_bc, in_=bv[:].partition_broadcast(P))
        nc.gpsimd.dma_start(out=bo_bc, in_=bo[:].partition_broadcast(P))
        for c in range(EC):
            nc.sync.dma_start(out=wv_sb[c][:, 0:512], in_=wv[c][:, 0:512])
            nc.sync.dma_start(out=wv_sb[c][:, 512:E], in_=wv[c][:, 512:E])
        for fb in range(FB):
            nc.sync.dma_start(
                out=wo_sb[:, fb],
                in_=wo[:, fb * 512 : (fb + 1) * 512].rearrange(
                    "(c p) f -> p c f", p=P
                ),
            )

        osbs = {0: {}, 1: {}}
        qts = {0: [], 1: []}
        kts = {0: [], 1: []}
        vts = {0: [], 1: []}
        ots = {0: [], 1: []}

        def emit_qk_ftile(g, which, ft):
            wt, bias_sb, lst = (
                (wq_sb[ft], bq_sb, qts[g])
                if which == "q"
                else (wk_sb[ft], bk_sb, kts[g])
            )
            ps = psmm.tile([P, N], F32, tag="mm")
            for c in range(EC):
                nc.tensor.matmul(
                    ps,
                    lhsT=wt[:, c, :],
                    rhs=xt_t[c][:, g * N : (g + 1) * N],
                    start=(c == 0),
                    stop=(c == EC - 1),
                )
            if which == "q":
                t = qtp.tile([P, N], FP16, tag="qt")
            else:
                t = ktp.tile([P, N], FP16, tag="kt")
            nc.vector.tensor_scalar(
                out=t, in0=ps, scalar1=bias_sb[:, ft : ft + 1], scalar2=None, op0=ADD
            )
            lst.append(t)

        def emit_v(g):
            # 4 token-chunk tiles [128, 64+1024]; ones block then feature
            # blocks of 512, each one accumulation chain + one bias add.
            for tt in range(NB):
                vt = vpp.tile([P, VW], FP16, tag="vp")
                for pr in range(EC):
                    nc.gpsimd.memset(vt[:, pr * PRW + DH : pr * PRW + 2 * DH], 1.0)
                vts[g].append(vt)

            def blocks4(base_ap, col0, blk_stride):
                b = base_ap[:, col0 : col0 + DH]
                return bass.AP(
                    tensor=b.tensor,
                    offset=b.offset,
                    ap=[list(b.ap[0]), [blk_stride, 4], [1, DH]],
                )

            for fb in range(FB):
                for tt in range(NB):
                    ps = psmm.tile([P, 512], F32, tag="mm")
                    for c in range(EC):
                        nc.tensor.matmul(
                            ps,
                            lhsT=xt_t[c][:, g * N + tt * P : g * N + (tt + 1) * P],
                            rhs=wv_sb[c][:, fb * 512 : (fb + 1) * 512],
                            start=(c == 0),
                            stop=(c == EC - 1),
                        )
                    # heads fb*8+hl live at pair blocks pr=fb*4+hl//2; even
                    # heads at block col 0, odd heads at col 128
                    for par in range(2):
                        nc.vector.tensor_add(
                            out=blocks4(
                                vts[g][tt], (fb * 4) * PRW + par * 2 * DH, PRW
                            ),
                            in0=blocks4(ps, par * DH, 2 * DH),
                            in1=blocks4(bv_bc, fb * 512 + par * DH, 2 * DH),
                        )

        def pv_lhsT(g, tt, h):
            # contiguous 128-col slice of the head's pair block: even h ->
            # {v_even | ones}; odd h -> {ones | v_odd}
            c0 = (h // 2) * PRW + (0 if h % 2 == 0 else DH)
            return vts[g][tt][:, c0 : c0 + P]

        def emit_attn_pair(g, pr):
            # scores for both heads of the pair, row-packed on the PE array
            # (K=64 each at array rows 0-63 / 64-127, separate PSUM banks)
            ex_AB = {}
            for h in (2 * pr, 2 * pr + 1):
                ex_AB[h] = expp.tile([P, NB, N], FP16, tag="exp", name=f"ex{h}")
            for half in range(2):
                scs = {}
                for h in (2 * pr, 2 * pr + 1):
                    lo, hi = (0, DH) if h % 2 == 0 else (DH, P)
                    sc = pssc.tile([P, 2, N], F32, tag="sc")
                    for cc in range(2):
                        c = 2 * half + cc
                        nc.tensor.matmul(
                            sc[:, cc],
                            lhsT=kts[g][pr][lo:hi, c * P : (c + 1) * P],
                            rhs=qts[g][pr][lo:hi, :],
                            start=True,
                            stop=True,
                        )
                    scs[h] = sc
                for h in (2 * pr, 2 * pr + 1):
                    nc.scalar.activation(
                        out=ex_AB[h][:, 2 * half : 2 * half + 2],
                        in_=scs[h],
                        func=EXP,
                    )
            # one [128,512] osb per pair: even head's o -> rows 0-63, odd
            # head's o -> rows 64-127 (same start partitions everywhere)
            osb = osbp.tile([P, N], FP16, tag="osb")
            for h in (2 * pr, 2 * pr + 1):
                ex = ex_AB[h]
                po = psop.tile([P, N], F32, tag="po")
                for c in range(NB):
                    nc.tensor.matmul(
                        po,
                        lhsT=pv_lhsT(g, c, h),
                        rhs=ex[:, c, :],
                        start=(c == 0),
                        stop=(c == NB - 1),
                    )
                # o half -> fp16 SBUF (DVE); one denominator row -> same-
                # partition stage (DVE), then DMA into the group's gather
                # tile (engines can't shift partitions; DMA can)
                lo = 0 if h % 2 == 0 else DH
                dn = DH if h % 2 == 0 else 0
                nc.vector.tensor_copy(
                    out=osb[lo : lo + DH, :], in_=po[lo : lo + DH, :]
                )
                den1 = recp.tile([P, N], F32, tag="den1")
                nc.vector.tensor_copy(
                    out=den1[dn : dn + 1, :], in_=po[dn : dn + 1, :]
                )
                nc.sync.dma_start(
                    out=denb[g][h : h + 1, :], in_=den1[dn : dn + 1, :]
                )
            osbs[g][pr] = osb

        def emit_recip_group(g):
            # one fast reciprocal for all 16 heads, then per-pair replication
            # DMA + normalize producing outproj lhsT tiles
            nc.vector.reciprocal_approx_fast(out=recb[g], in_=denb[g])
            for pr in range(EC):
                ot = otp.tile([P, N], FP16, tag="ot")
                rep = recp.tile([P, N], F32, tag="rep")
                for hl in range(2):
                    h = 2 * pr + hl
                    s = recb[g][h : h + 1, :]
                    nc.sync.dma_start(
                        out=rep[hl * DH : (hl + 1) * DH, :],
                        in_=bass.AP(
                            tensor=s.tensor,
                            offset=s.offset,
                            ap=[list(s.ap[0]), [0, DH], list(s.ap[1])],
                        ),
                    )
                    nc.vector.tensor_mul(
                        out=ot[hl * DH : (hl + 1) * DH, :],
                        in0=osbs[g][pr][hl * DH : (hl + 1) * DH, :],
                        in1=rep[hl * DH : (hl + 1) * DH, :],
                    )
                ots[g].append(ot)

        def emit_outproj_unit(g, u, last=False):
            fb, tt = u // NB, u % NB
            ps = psmm.tile([P, 512], F32, tag="mm")
            for dc in range(EC):
                nc.tensor.matmul(
                    ps,
                    lhsT=ots[g][dc][:, tt * P : (tt + 1) * P],
                    rhs=wo_sb[:, fb, dc, :],
                    start=(dc == 0),
                    stop=(dc == EC - 1),
                )
            ob = outp.tile([P, 512], FP16, tag="ob")
            nc.vector.tensor_add(
                out=ob, in0=ps, in1=bo_bc[:, fb * 512 : (fb + 1) * 512]
            )
            nsplit = 4 if last else 2
            step = P // nsplit
            r0 = g * N + tt * P
            for i in range(nsplit):
                nc.sync.dma_start(
                    out=out[
                        r0 + i * step : r0 + (i + 1) * step,
                        fb * 512 : (fb + 1) * 512,
                    ],
                    in_=ob[i * step : (i + 1) * step, :],
                )

        # ---- software-pipelined program order --------------------------
        for ft in range(EC):
            emit_qk_ftile(0, "q", ft)
            emit_qk_ftile(0, "k", ft)
        emit_v(0)
        # group-0 attention interleaved with group-1 q/k proj
        for pr in range(EC):
            emit_attn_pair(0, pr)
            emit_qk_ftile(1, "q", pr)
            emit_qk_ftile(1, "k", pr)
        emit_recip_group(0)  # DVE/DMA only; PE proceeds straight to v(1)
        emit_v(1)
        # group-1 attention interleaved with group-0 out proj units 0-5;
        # units 6,7 held back to cover the PE during recip(1)
        for pr in range(EC):
            emit_attn_pair(1, pr)
            if 2 <= pr:
                emit_outproj_unit(0, pr - 2)
        emit_recip_group(1)
        emit_outproj_unit(0, 6)
        emit_outproj_unit(0, 7)
        for u in range(2 * NB):
            emit_outproj_unit(1, u, last=(u == 2 * NB - 1))
    nc.finalize()
    return nc


def _get_nc():
    if "nc" not in _CACHE:
        _CACHE["nc"] = _build_nc()
    return _CACHE["nc"]


def _make_in_maps(x, Wqkv, bqkv, Wout, bout):
    """Host-side sharding: permute tokens to group-major, pre-transpose x."""
    x = np.asarray(x, dtype=np.float32)
    Wqkv = np.asarray(Wqkv, dtype=np.float32)
    bqkv = np.asarray(bqkv, dtype=np.float32)
    Wout = np.ascontiguousarray(np.asarray(Wout, dtype=np.float16))
    bout = np.ascontiguousarray(np.asarray(bout, dtype=np.float32))

    # group-major token order: x_perm[b, g*N + i] = x[b, i*ST + g]
    x_perm = x.reshape(B, N, ST, E).transpose(0, 2, 1, 3)  # [B, ST, N, E]

    # [E, E] -> [ft, p, c, f] tile-major so each SBUF partition reads big runs
    def tile_qk(w):
        return np.ascontiguousarray(
            w.reshape(EC, P, EC, P).transpose(2, 1, 0, 3).astype(np.float16)
        )

    wq = tile_qk(Wqkv[:, 0:E] * SCALE)
    wk = tile_qk(Wqkv[:, E : 2 * E])
    wv = np.ascontiguousarray(
        Wqkv[:, 2 * E : 3 * E].astype(np.float16).reshape(EC, P, E)
    )
    bq = np.ascontiguousarray(bqkv[0:E] * SCALE)
    bk = np.ascontiguousarray(bqkv[E : 2 * E])
    bv = np.ascontiguousarray(bqkv[2 * E : 3 * E])

    in_maps = []
    for c in range(NCORES):
        b = c // (NCORES // B)
        g0 = GPC * (c % (NCORES // B))
        xc = x_perm[b, g0 : g0 + GPC].reshape(TOK, E)  # [1024, E]
        xct = np.ascontiguousarray(xc.T.astype(np.float16))  # [E, 1024]
        in_maps.append(
            {
                "xt": xct,
                "wq": wq,
                "wk": wk,
                "wv": wv,
                "wo": Wout,
                "bq": bq,
                "bk": bk,
                "bv": bv,
                "bo": bout,
            }
        )
    return in_maps


def kernel(x, Wqkv, bqkv, Wout, bout):
    from concourse.bass_utils import run_bass_kernel_spmd

    nc = _get_nc()
    in_maps = _make_in_maps(x, Wqkv, bqkv, Wout, bout)
    trace = bool(int(os.environ.get("KERNEL_TRACE", "0")))
    res = run_bass_kernel_spmd(
        nc, in_maps, core_ids=list(range(NCORES)), trace=trace
    )
    _CACHE["last_result"] = res

    # reassemble: core outputs are [1024 tok, E] fp16 in group-major order
    out = np.empty((B, S, E), dtype=np.float32)
    for b in range(B):
        per_b = [
            res.results[b * (NCORES // B) + j]["out"].astype(np.float32)
            for j in range(NCORES // B)
        ]
        perm = np.concatenate(per_b, axis=0)  # [ST*N, E] group-major
        out[b] = perm.reshape(ST, N, E).transpose(1, 0, 2).reshape(S, E)
    return out


# revision 19
# speedup vs baseline: 1.3205x; 1.3205x over previous
"""Strided (residue-group) attention for Trainium2, SPMD across 8 NeuronCores.

Problem: x[B=2,S=4096,E=1024] -> qkv proj -> per-(batch,head,residue-group)
attention (stride 8 -> 8 groups of n=512 tokens) -> out proj.

Sharding: by (batch, residue-group).  B*stride = 16 group-instances; each of
the 8 cores owns 2 (batch,group) pairs = 1024 tokens and computes their FULL
output rows (it holds all 16 heads for its tokens).  The residue groups are
independent, so there are no cross-device collectives at all; the host
permutes tokens into group-major order on the way in and inverts on the way
out.

Device kernel design (per core), v2:
  - All weights (wq/wk/wv/wo) SBUF-resident in fp16; startup DMAs split into
    <=128KB pieces so they spread across the 16 DMA queues.
  - QKV: qT,kT produced feature-on-partition ([f,tok]); v produced
    token-on-partition ([tok,f]).
  - v tiles [128tok, 8 pairs x 192]: per head-pair block {v_even(64) |
    ones(64) | v_odd(64)} with the ones written once by memset.  PV lhsT is
    a contiguous 128-col slice: cols 0-127 for the even head (o lands at
    PSUM rows 0-63), cols 64-191 for the odd head (o at rows 64-127) --
    matching the outproj lhsT row layout with no cross-partition moves,
    while 64 copies of the softmax denominator fill the other row half for
    free (matmul cost only depends on the moving free size).
  - scoresT[k,q] = kT.T-chunks @ qT per head; head pairs are row-packed on
    the PE array (K=64 each at array rows 0-63 / 64-127).
  - exp on ScalarE without max-subtraction (scores are O(+-8), exp is safe).
    EXP is the only table the ScalarE ever loads (no ln-based reciprocal
    anymore -> no activation-table thrash).
  - softmax denominators: one row per head gathered (ScalarE copy) into a
    [16, 512] f32 tile per group; ONE nc.vector.reciprocal_approx_fast per
    group; replication DMA spreads each recip row across 64 partitions; DVE
    multiplies normalize o.
  - out proj: lhsT = oT chunks, rhs = resident Wout rows -> [tok, E] fp16
    output, DMA'd out in partition-split pieces (4-way for the final unit so
    the tail DMA is short).
"""

import os

import numpy as np

B, S, E = 2, 4096, 1024
H, ST = 16, 8
DH = E // H  # 64
N = S // ST  # 512 tokens per residue group
NCORES = 8
GPC = (B * ST) // NCORES  # 2 (batch,group) pairs per core
TOK = GPC * N  # 1024 tokens per core
P = 128
EC = E // P  # 8 contraction chunks of 128
NB = N // P  # 4 token chunks of 128 per group
FB = 2  # feature blocks of 512 in E
SCALE = 1.0 / float(np.sqrt(DH))

_CACHE: dict = {}


def _build_nc():
    import concourse.bass as bass
    import concourse.bacc as bacc
    import concourse.tile as tile
    from concourse import mybir

    F32 = mybir.dt.float32
    FP16 = mybir.dt.float16
    ADD = mybir.AluOpType.add
    EXP = mybir.ActivationFunctionType.Exp

    PRW = 3 * DH  # per-pair v block width: v_even | ones | v_odd
    VW = EC * PRW  # v tile free width (8 pairs x 192)

    nc = bacc.Bacc()
    xt = nc.declare_dram_parameter("xt", [E, TOK], FP16, isOutput=False)
    wq = nc.declare_dram_parameter("wq", [EC, P, EC, P], FP16, isOutput=False)
    wk = nc.declare_dram_parameter("wk", [EC, P, EC, P], FP16, isOutput=False)
    wv = nc.declare_dram_parameter("wv", [EC, P, E], FP16, isOutput=False)
    wo = nc.declare_dram_parameter("wo", [E, E], FP16, isOutput=False)
    bq = nc.declare_dram_parameter("bq", [E], F32, isOutput=False)
    bk = nc.declare_dram_parameter("bk", [E], F32, isOutput=False)
    bv = nc.declare_dram_parameter("bv", [E], F32, isOutput=False)
    bo = nc.declare_dram_parameter("bo", [E], F32, isOutput=False)
    out = nc.declare_dram_parameter("out", [TOK, E], FP16, isOutput=True)

    with tile.TileContext(nc) as tc, (
        tc.tile_pool(name="const", bufs=1)
    ) as const, tc.tile_pool(name="xtp", bufs=1) as xtp, tc.tile_pool(
        name="wqp", bufs=1
    ) as wqp, tc.tile_pool(name="wkp", bufs=1) as wkp, tc.tile_pool(
        name="wvp", bufs=1
    ) as wvp, tc.tile_pool(name="qtp", bufs=9) as qtp, tc.tile_pool(
        name="ktp", bufs=9
    ) as ktp, tc.tile_pool(name="vpp", bufs=8) as vpp, tc.tile_pool(
        name="expp", bufs=4
    ) as expp, tc.tile_pool(name="otp", bufs=16) as otp, tc.tile_pool(
        name="recp", bufs=4
    ) as recp, tc.tile_pool(name="outp", bufs=3) as outp, tc.tile_pool(
        name="osbp", bufs=18
    ) as osbp, tc.tile_pool(name="psmm", bufs=2, space="PSUM") as psmm, tc.tile_pool(
        name="pssc", bufs=2, space="PSUM"
    ) as pssc, tc.tile_pool(name="pso", bufs=2, space="PSUM") as psop:
        # ---- persistent tiles ------------------------------------------
        bq_sb = const.tile([P, EC], F32)
        bk_sb = const.tile([P, EC], F32)
        bv_bc = const.tile([P, E], F32)
        bo_bc = const.tile([P, E], F32)
        wo_sb = const.tile([P, FB, EC, 512], FP16)
        wq_sb = [wqp.tile([P, EC, P], FP16, name=f"wq{ft}") for ft in range(EC)]
        wk_sb = [wkp.tile([P, EC, P], FP16, name=f"wk{ft}") for ft in range(EC)]
        wv_sb = [wvp.tile([P, E], FP16, name=f"wv{c}") for c in range(EC)]
        xt_t = [xtp.tile([P, TOK], FP16, name=f"xt{c}") for c in range(EC)]
        denb = [const.tile([H, N], F32, name=f"denb{g}") for g in range(GPC)]
        recb = [const.tile([H, N], F32, name=f"recb{g}") for g in range(GPC)]
        recb16 = [const.tile([H, N], FP16, name=f"recb16_{g}") for g in range(GPC)]

        # ---- startup DMAs (ordered for earliest first matmul; dispatches
        # alternate between the two hardware DGE queues: SP and Activation)
        _dq = [0]

        def ld(out, in_):
            eng = nc.sync if (_dq[0] % 2 == 0) else nc.scalar
            _dq[0] += 1
            eng.dma_start(out=out, in_=in_)

        ld(bq_sb, bq[:].rearrange("(c p) -> p c", p=P))
        ld(bk_sb, bk[:].rearrange("(c p) -> p c", p=P))
        # wq ftile 0 split in quarters, then group-0 xt halves: these gate
        # the first q-projection chain.
        for q4 in range(4):
            ld(wq_sb[0][:, 2 * q4 : 2 * q4 + 2, :], wq[0][:, 2 * q4 : 2 * q4 + 2, :])
        for c in range(EC):
            ld(xt_t[c][:, 0:N], xt[c * P : (c + 1) * P, 0:N])
        ld(wk_sb[0][:, 0:4, :], wk[0][:, 0:4, :])
        ld(wk_sb[0][:, 4:8, :], wk[0][:, 4:8, :])
        for ft in range(1, EC):
            ld(wq_sb[ft][:, 0:4, :], wq[ft][:, 0:4, :])
            ld(wq_sb[ft][:, 4:8, :], wq[ft][:, 4:8, :])
            ld(wk_sb[ft][:, 0:4, :], wk[ft][:, 0:4, :])
            ld(wk_sb[ft][:, 4:8, :], wk[ft][:, 4:8, :])
        for c in range(EC):
            ld(xt_t[c][:, N:TOK], xt[c * P : (c + 1) * P, N:TOK])
        nc.gpsimd.dma_start(out=bv_bc, in_=bv[:].partition_broadcast(P))
        nc.gpsimd.dma_start(out=bo_bc, in_=bo[:].partition_broadcast(P))
        for c in range(EC):
            ld(wv_sb[c][:, 0:512], wv[c][:, 0:512])
            ld(wv_sb[c][:, 512:E], wv[c][:, 512:E])
        for fb in range(FB):
            ld(
                wo_sb[:, fb],
                wo[:, fb * 512 : (fb + 1) * 512].rearrange(
                    "(c p) f -> p c f", p=P
                ),
            )

        osbs = {0: {}, 1: {}}
        qts = {0: [], 1: []}
        kts = {0: [], 1: []}
        vts = {0: [], 1: []}
        ots = {0: [], 1: []}

        def emit_qk_ftile(g, which, ft):
            wt, bias_sb, lst = (
                (wq_sb[ft], bq_sb, qts[g])
                if which == "q"
                else (wk_sb[ft], bk_sb, kts[g])
            )
            ps = psmm.tile([P, N], F32, tag="mm")
            for c in range(EC):
                nc.tensor.matmul(
                    ps,
                    lhsT=wt[:, c, :],
                    rhs=xt_t[c][:, g * N : (g + 1) * N],
                    start=(c == 0),
                    stop=(c == EC - 1),
                )
            if which == "q":
                t = qtp.tile([P, N], FP16, tag="qt")
            else:
                t = ktp.tile([P, N], FP16, tag="kt")
            nc.vector.tensor_scalar(
                out=t, in0=ps, scalar1=bias_sb[:, ft : ft + 1], scalar2=None, op0=ADD
            )
            lst.append(t)

        def emit_v(g):
            # 4 token-chunk tiles [128, 64+1024]; ones block then feature
            # blocks of 512, each one accumulation chain + one bias add.
            for tt in range(NB):
                vt = vpp.tile([P, VW], FP16, tag="vp")
                for pr in range(EC):
                    nc.gpsimd.memset(vt[:, pr * PRW + DH : pr * PRW + 2 * DH], 1.0)
                vts[g].append(vt)

            def blocks4(base_ap, col0, blk_stride):
                b = base_ap[:, col0 : col0 + DH]
                return bass.AP(
                    tensor=b.tensor,
                    offset=b.offset,
                    ap=[list(b.ap[0]), [blk_stride, 4], [1, DH]],
                )

            for fb in range(FB):
                for tt in range(NB):
                    ps = psmm.tile([P, 512], F32, tag="mm")
                    for c in range(EC):
                        nc.tensor.matmul(
                            ps,
                            lhsT=xt_t[c][:, g * N + tt * P : g * N + (tt + 1) * P],
                            rhs=wv_sb[c][:, fb * 512 : (fb + 1) * 512],
                            start=(c == 0),
                            stop=(c == EC - 1),
                        )
                    # heads fb*8+hl live at pair blocks pr=fb*4+hl//2; even
                    # heads at block col 0, odd heads at col 128
                    for par in range(2):
                        nc.vector.tensor_add(
                            out=blocks4(
                                vts[g][tt], (fb * 4) * PRW + par * 2 * DH, PRW
                            ),
                            in0=blocks4(ps, par * DH, 2 * DH),
                            in1=blocks4(bv_bc, fb * 512 + par * DH, 2 * DH),
                        )

        def pv_lhsT(g, tt, h):
            # contiguous 128-col slice of the head's pair block: even h ->
            # {v_even | ones}; odd h -> {ones | v_odd}
            c0 = (h // 2) * PRW + (0 if h % 2 == 0 else DH)
            return vts[g][tt][:, c0 : c0 + P]

        def emit_attn_pair(g, pr):
            # scores for both heads of the pair, row-packed on the PE array
            # (K=64 each at array rows 0-63 / 64-127, separate PSUM banks)
            ex_AB = {}
            for h in (2 * pr, 2 * pr + 1):
                ex_AB[h] = expp.tile([P, NB, N], FP16, tag="exp", name=f"ex{h}")
            for half in range(2):
                scs = {}
                for h in (2 * pr, 2 * pr + 1):
                    lo, hi = (0, DH) if h % 2 == 0 else (DH, P)
                    sc = pssc.tile([P, 2, N], F32, tag="sc")
                    for cc in range(2):
                        c = 2 * half + cc
                        nc.tensor.matmul(
                            sc[:, cc],
                            lhsT=kts[g][pr][lo:hi, c * P : (c + 1) * P],
                            rhs=qts[g][pr][lo:hi, :],
                            start=True,
                            stop=True,
                        )
                    scs[h] = sc
                for h in (2 * pr, 2 * pr + 1):
                    nc.scalar.activation(
                        out=ex_AB[h][:, 2 * half : 2 * half + 2],
                        in_=scs[h],
                        func=EXP,
                    )
            # one [128,512] osb per pair: even head's o -> rows 0-63, odd
            # head's o -> rows 64-127 (same start partitions everywhere)
            osb = osbp.tile([P, N], FP16, tag="osb")
            for h in (2 * pr, 2 * pr + 1):
                ex = ex_AB[h]
                po = psop.tile([P, N], F32, tag="po")
                for c in range(NB):
                    nc.tensor.matmul(
                        po,
                        lhsT=pv_lhsT(g, c, h),
                        rhs=ex[:, c, :],
                        start=(c == 0),
                        stop=(c == NB - 1),
                    )
                # o half -> fp16 SBUF (DVE); one denominator row -> same-
                # partition stage (DVE), then DMA into the group's gather
                # tile (engines can't shift partitions; DMA can)
                lo = 0 if h % 2 == 0 else DH
                dn = DH if h % 2 == 0 else 0
                nc.vector.tensor_copy(
                    out=osb[lo : lo + DH, :], in_=po[lo : lo + DH, :]
                )
                den1 = recp.tile([P, N], F32, tag="den1")
                nc.vector.tensor_copy(
                    out=den1[dn : dn + 1, :], in_=po[dn : dn + 1, :]
                )
                nc.sync.dma_start(
                    out=denb[g][h : h + 1, :], in_=den1[dn : dn + 1, :]
                )
            osbs[g][pr] = osb

        def emit_recip_group(g):
            # one fast reciprocal for all 16 heads, then per-pair replication
            # DMA + normalize producing outproj lhsT tiles
            nc.vector.reciprocal_approx_fast(out=recb[g], in_=denb[g])
            nc.vector.tensor_copy(out=recb16[g], in_=recb[g])
            for pr in range(EC):
                ot = otp.tile([P, N], FP16, tag="ot")
                rep = recp.tile([P, N], FP16, tag="rep")
                # one replication DMA per pair (both heads' recip rows, each
                # spread across 64 partitions), on the idle GpSimd queue so
                # the SP queue keeps servicing output DMAs
                s = recb16[g][2 * pr : 2 * pr + 2, :]
                nc.gpsimd.dma_start(
                    out=rep,
                    in_=bass.AP(
                        tensor=s.tensor,
                        offset=s.offset,
                        ap=[list(s.ap[0]), [0, DH], list(s.ap[1])],
                    ),
                )
                for hl in range(2):
                    nc.vector.tensor_mul(
                        out=ot[hl * DH : (hl + 1) * DH, :],
                        in0=osbs[g][pr][hl * DH : (hl + 1) * DH, :],
                        in1=rep[hl * DH : (hl + 1) * DH, :],
                    )
                ots[g].append(ot)

        def emit_outproj_unit(g, u, last=False):
            fb, tt = u // NB, u % NB
            ps = psmm.tile([P, 512], F32, tag="mm")
            for dc in range(EC):
                nc.tensor.matmul(
                    ps,
                    lhsT=ots[g][dc][:, tt * P : (tt + 1) * P],
                    rhs=wo_sb[:, fb, dc, :],
                    start=(dc == 0),
                    stop=(dc == EC - 1),
                )
            ob = outp.tile([P, 512], FP16, tag="ob")
            nc.vector.tensor_add(
                out=ob, in0=ps, in1=bo_bc[:, fb * 512 : (fb + 1) * 512]
            )
            nsplit = 4 if last else 2
            step = P // nsplit
            r0 = g * N + tt * P
            for i in range(nsplit):
                nc.sync.dma_start(
                    out=out[
                        r0 + i * step : r0 + (i + 1) * step,
                        fb * 512 : (fb + 1) * 512,
                    ],
                    in_=ob[i * step : (i + 1) * step, :],
                )

        # ---- software-pipelined program order --------------------------
        for ft in range(EC):
            emit_qk_ftile(0, "q", ft)
            emit_qk_ftile(0, "k", ft)
        emit_v(0)
        # group-0 attention interleaved with group-1 q/k proj
        for pr in range(EC):
            emit_attn_pair(0, pr)
            emit_qk_ftile(1, "q", pr)
            emit_qk_ftile(1, "k", pr)
        # v(1) first so its DVE bias-adds precede the recip muls in the
        # in-order DVE queue (the PE streams v(1) during recip(0))
        emit_v(1)
        emit_recip_group(0)
        # group-1 attention; group-0 outproj units 0-3 interleave late
        # (their ot(0) inputs trail the v(1) adds on the DVE queue), units
        # 4-7 held back to cover the PE during recip(1)
        for pr in range(EC):
            emit_attn_pair(1, pr)
            if 4 <= pr:
                emit_outproj_unit(0, pr - 4)
        emit_recip_group(1)
        for u in range(4, 2 * NB):
            emit_outproj_unit(0, u)
        for u in range(2 * NB):
            emit_outproj_unit(1, u, last=(u == 2 * NB - 1))
    nc.finalize()
    return nc


def _get_nc():
    if "nc" not in _CACHE:
        _CACHE["nc"] = _build_nc()
    return _CACHE["nc"]


def _make_in_maps(x, Wqkv, bqkv, Wout, bout):
    """Host-side sharding: permute tokens to group-major, pre-transpose x."""
    x = np.asarray(x, dtype=np.float32)
    Wqkv = np.asarray(Wqkv, dtype=np.float32)
    bqkv = np.asarray(bqkv, dtype=np.float32)
    Wout = np.ascontiguousarray(np.asarray(Wout, dtype=np.float16))
    bout = np.ascontiguousarray(np.asarray(bout, dtype=np.float32))

    # group-major token order: x_perm[b, g*N + i] = x[b, i*ST + g]
    x_perm = x.reshape(B, N, ST, E).transpose(0, 2, 1, 3)  # [B, ST, N, E]

    # [E, E] -> [ft, p, c, f] tile-major so each SBUF partition reads big runs
    def tile_qk(w):
        return np.ascontiguousarray(
            w.reshape(EC, P, EC, P).transpose(2, 1, 0, 3).astype(np.float16)
        )

    wq = tile_qk(Wqkv[:, 0:E] * SCALE)
    wk = tile_qk(Wqkv[:, E : 2 * E])
    wv = np.ascontiguousarray(
        Wqkv[:, 2 * E : 3 * E].astype(np.float16).reshape(EC, P, E)
    )
    bq = np.ascontiguousarray(bqkv[0:E] * SCALE)
    bk = np.ascontiguousarray(bqkv[E : 2 * E])
    bv = np.ascontiguousarray(bqkv[2 * E : 3 * E])

    in_maps = []
    for c in range(NCORES):
        b = c // (NCORES // B)
        g0 = GPC * (c % (NCORES // B))
        xc = x_perm[b, g0 : g0 + GPC].reshape(TOK, E)  # [1024, E]
        xct = np.ascontiguousarray(xc.T.astype(np.float16))  # [E, 1024]
        in_maps.append(
            {
                "xt": xct,
                "wq": wq,
                "wk": wk,
                "wv": wv,
                "wo": Wout,
                "bq": bq,
                "bk": bk,
                "bv": bv,
                "bo": bout,
            }
        )
    return in_maps


def kernel(x, Wqkv, bqkv, Wout, bout):
    from concourse.bass_utils import run_bass_kernel_spmd

    nc = _get_nc()
    in_maps = _make_in_maps(x, Wqkv, bqkv, Wout, bout)
    trace = bool(int(os.environ.get("KERNEL_TRACE", "0")))
    res = run_bass_kernel_spmd(
        nc, in_maps, core_ids=list(range(NCORES)), trace=trace
    )
    _CACHE["last_result"] = res

    # reassemble: core outputs are [1024 tok, E] fp16 in group-major order
    out = np.empty((B, S, E), dtype=np.float32)
    for b in range(B):
        per_b = [
            res.results[b * (NCORES // B) + j]["out"].astype(np.float32)
            for j in range(NCORES // B)
        ]
        perm = np.concatenate(per_b, axis=0)  # [ST*N, E] group-major
        out[b] = perm.reshape(ST, N, E).transpose(1, 0, 2).reshape(S, E)
    return out
